# revision 1
# baseline (speedup 1.0000x reference)
"""Trainium2 Bass kernel for the SNN (LIF) network:

    cur1 = x.reshape(B,-1) @ W1.T + b1          (big fp32 matmul, once)
    200 sequential LIF steps on [B,1000] (layer 1), tiny matmul into 5
    outputs per step (layer 2), second LIF on [B,5].

Distribution over 8 cores:
  Phase A: contraction(K)-sharded exact-fp32 matmul -> per-core partial cur1
           [256, 1024(padded)], ReduceScatter(add) -> each core owns a
           32-row batch slice of cur1.
  Phase B: per-core LIF layer-1 scan over its 32-batch slice, hidden on
           partitions ([128, 8chunks x 32batch] tiles). One custom DVE
           instruction per step: mem' = beta*mem + cur - (mem > 1).
           ACT computes g = Sign(mem - 1) in bf16 (spk = (1+g)/2 folded
           into W2/b2 on the host).
  Phase C: every 4 steps, PE contracts g (stationary [128, 4*32]) against
           W2 chunks split hi/lo bf16 (exact), + bias matmul, into PSUM.
  Phase D: layer-2 LIF scan on [32, 5] per step; spk2 = (mem2 > 1) at the
           end. Outputs gathered on host.
"""
import sys

if "/opt/trn_rl_repo" not in sys.path:
    sys.path.insert(0, "/opt/trn_rl_repo")

import numpy as np
import ml_dtypes

# ---------------------------------------------------------------- constants
BETA = 0.95
T = 200
B = 256
NIN = 32000
NH = 1000
NO = 5

N_CORES = 8
KPAD = 32768           # NIN padded to 256*128
KC = KPAD // N_CORES   # 4096 contraction per core
KTILES = KC // 128     # 32
HPAD = 1024            # hidden padded
BLOC = B // N_CORES    # 32 batch rows per core after ReduceScatter
NCHUNK = HPAD // 128   # 8 hidden chunks of 128
G = 4                  # phase-C group size (steps per PE batch)
NGROUP = T // G        # 50
W1SCALE = 256.0        # W1 pre-scale so the fp16 lo-half stays normal

# ---------------------------------------------------------------- custom op
_LIF_NAME = "LIF_STEP_ANT"


def _register_lif_op():
    from concourse.dve_ops import (
        DveOp, OPS, CUSTOM_DVE_SPECS, _SUB_OPCODE_FOR_NAME, _CUSTOM_DVE_ROW_BASE,
    )
    from concourse.dve_spec import Spec, Src0, Src1, C0, One, lower as dve_lower, _has_src1
    from concourse.dve_uop import DveOpSpec

    for op in OPS:
        if op.name == _LIF_NAME:
            return op
    spec = Spec(
        body=Src0 * C0 + Src1 - (Src0 > One),
        reference=lambda in0, in1, s0: in0 * s0 + in1 - (in0 > 1.0).astype(np.float32),
    )
    if _LIF_NAME not in _SUB_OPCODE_FOR_NAME:
        _SUB_OPCODE_FOR_NAME[_LIF_NAME] = _CUSTOM_DVE_ROW_BASE + len(OPS)
    shas = {}
    for ver in ("v3", "v4"):
        s = DveOpSpec(
            name=_LIF_NAME,
            opcode=_SUB_OPCODE_FOR_NAME[_LIF_NAME],
            uops=dve_lower(spec, ver=ver),
            rd1_en=_has_src1(spec),
        )
        shas[ver] = s.sha(ver)
    op = DveOp(_LIF_NAME, spec, subdim=False, uops_sha=shas)
    OPS.append(op)
    CUSTOM_DVE_SPECS[_LIF_NAME] = op.spec
    return op


# ---------------------------------------------------------------- program
_PROGRAMS = {}  # sim -> (nc, lif_op)


def _build_program(sim=False):
    if sim in _PROGRAMS:
        return _PROGRAMS[sim]

    import concourse.bass as bass
    import concourse.tile as tile
    from concourse import bacc, mybir
    from concourse.masks import make_identity

    LIF = _register_lif_op()
    f32 = mybir.dt.float32
    bf16 = mybir.dt.bfloat16

    nc = bacc.Bacc("TRN2", target_bir_lowering=False, debug=False,
                   num_devices=1 if sim else N_CORES)

    f16 = mybir.dt.float16
    # inputs (per-core)
    xth_d = nc.dram_tensor("xth", [KTILES, 128, B], f16, kind="ExternalInput").ap()
    xtl_d = nc.dram_tensor("xtl", [KTILES, 128, B], f16, kind="ExternalInput").ap()
    w1h_d = nc.dram_tensor("w1h", [KTILES, 128, HPAD], f16, kind="ExternalInput").ap()
    w1l_d = nc.dram_tensor("w1l", [KTILES, 128, HPAD], f16, kind="ExternalInput").ap()
    b1c_d = nc.dram_tensor("b1c", [128, NCHUNK], f32, kind="ExternalInput").ap()
    w2hi_d = nc.dram_tensor("w2hi", [128, NCHUNK, NO], bf16, kind="ExternalInput").ap()
    w2lo_d = nc.dram_tensor("w2lo", [128, NCHUNK, NO], bf16, kind="ExternalInput").ap()
    b2e_d = nc.dram_tensor("b2e", [1, NO], f32, kind="ExternalInput").ap()
    # outputs (per-core batch slice), free layout = (t, o)
    mem2_d = nc.dram_tensor("mem2rec", [BLOC, T * NO], f32, kind="ExternalOutput").ap()
    spk2_d = nc.dram_tensor("spk2rec", [BLOC, T * NO], f32, kind="ExternalOutput").ap()
    curdbg_d = nc.dram_tensor("curdbg", [BLOC, HPAD], f32, kind="ExternalOutput").ap()

    with tile.TileContext(nc) as tc:
        with (
            tc.tile_pool(name="kin", bufs=3) as kpool,
            tc.tile_pool(name="win", bufs=3) as wpool,
            tc.tile_pool(name="psA", bufs=1, space="PSUM") as psA,
            tc.tile_pool(name="stage", bufs=1) as stage,
            tc.tile_pool(name="dram", bufs=1, space="DRAM") as dram,
            tc.tile_pool(name="mem", bufs=3) as mpool,
            tc.tile_pool(name="g4", bufs=3) as gpool,
            tc.tile_pool(name="psC", bufs=2, space="PSUM") as psC,
            tc.tile_pool(name="psT", bufs=2, space="PSUM") as psT,
        ):
            # ---------------- phase A: cur1 partial = xT_slice.T @ W1T_slice
            ps = [[psA.tile([128, 512], f32, tag=f"ps{mb}{nb}", name=f"ps{mb}{nb}")
                   for nb in range(2)] for mb in range(2)]
            for kt in range(KTILES):
                xh_t = kpool.tile([128, B], f16, tag="xth")
                nc.sync.dma_start(xh_t[:], xth_d[kt])
                xl_t = kpool.tile([128, B], f16, tag="xtl")
                nc.sync.dma_start(xl_t[:], xtl_d[kt])
                wh_t = wpool.tile([128, HPAD], f16, tag="w1h")
                nc.sync.dma_start(wh_t[:], w1h_d[kt])
                wl_t = wpool.tile([128, HPAD], f16, tag="w1l")
                nc.sync.dma_start(wl_t[:], w1l_d[kt])
                last = kt == KTILES - 1
                for mb in range(2):
                    xh_s = xh_t[:, mb * 128:(mb + 1) * 128]
                    xl_s = xl_t[:, mb * 128:(mb + 1) * 128]
                    # keep each stationary operand loaded across streams
                    for nb in range(2):
                        out = ps[mb][nb][:]
                        nc.tensor.matmul(out, xh_s, wl_t[:, nb * 512:(nb + 1) * 512],
                                         start=(kt == 0), stop=False)
                        nc.tensor.matmul(out, xh_s, wh_t[:, nb * 512:(nb + 1) * 512],
                                         start=False, stop=False)
                    for nb in range(2):
                        nc.tensor.matmul(ps[mb][nb][:], xl_s,
                                         wh_t[:, nb * 512:(nb + 1) * 512],
                                         start=False, stop=last)
            partial = dram.tile([B, HPAD], f32)
            for mb in range(2):
                cs = stage.tile([128, HPAD], f32, tag=f"curp{mb}")
                nc.scalar.activation(cs[:, 0:512], ps[mb][0][:],
                                     mybir.ActivationFunctionType.Copy, scale=1.0 / W1SCALE)
                nc.scalar.activation(cs[:, 512:1024], ps[mb][1][:],
                                     mybir.ActivationFunctionType.Copy, scale=1.0 / W1SCALE)
                nc.sync.dma_start(partial[mb * 128:(mb + 1) * 128, :], cs[:])

            # ---------------- ReduceScatter: each core gets its 32-batch slice
            rs_out = dram.tile([BLOC, HPAD], f32)
            if sim:
                # timing stand-in for the collective (single-core TimelineSim)
                nc.sync.dma_start(rs_out[:], partial[0:BLOC, :])
            else:
                nc.gpsimd.collective_compute(
                    "ReduceScatter",
                    mybir.AluOpType.add,
                    replica_groups=[list(range(N_CORES))],
                    ins=[partial.opt()],
                    outs=[rs_out.opt()],
                )

            # ---------------- transpose to scan layout + fold b1
            # curb[p, c*32 + b] = cur1[b, c*128 + p] + b1[c*128 + p]
            rsb = stage.tile([BLOC, HPAD], f32, tag="rsb")
            nc.sync.dma_start(rsb[:], rs_out[:])
            nc.sync.dma_start(curdbg_d[:], rsb[:])
            ident = stage.tile([BLOC, BLOC], f32, tag="ident")
            make_identity(nc, ident[:])
            b1t = stage.tile([128, NCHUNK], f32, tag="b1t")
            nc.sync.dma_start(b1t[:], b1c_d[:])
            curb = stage.tile([128, NCHUNK * BLOC], f32, tag="curb")
            for c in range(NCHUNK):
                pt = psT.tile([128, BLOC], f32, tag="pst")
                nc.tensor.transpose(pt[:], rsb[:, c * 128:(c + 1) * 128], ident[:])
                nc.scalar.activation(
                    curb[:, c * BLOC:(c + 1) * BLOC], pt[:],
                    mybir.ActivationFunctionType.Identity,
                    bias=b1t[:, c:c + 1], scale=1.0,
                )

            # ---------------- scan constants
            w2hi_t = stage.tile([128, NCHUNK, NO], bf16, tag="w2hi")
            nc.sync.dma_start(w2hi_t[:], w2hi_d[:])
            w2lo_t = stage.tile([128, NCHUNK, NO], bf16, tag="w2lo")
            nc.sync.dma_start(w2lo_t[:], w2lo_d[:])
            b2e_t = stage.tile([1, NO], f32, tag="b2e")
            nc.sync.dma_start(b2e_t[:], b2e_d[:])
            ones_t = stage.tile([1, 128], f32, tag="ones")
            nc.vector.memset(ones_t[:], 1.0)
            biasm1 = stage.tile([128, 1], f32, tag="bm1")
            nc.vector.memset(biasm1[:], -1.0)
            zeros_t = stage.tile([128, NCHUNK * BLOC], f32, tag="zeros")
            nc.vector.memset(zeros_t[:], 0.0)
            cur2buf = stage.tile([128, NGROUP * NO], f32, tag="cur2buf")

            # ---------------- phase B/C: layer-1 scan + layer-2 matmul
            mem_prev = zeros_t
            gt = None
            for t in range(1, T + 1):
                gi, sl = (t - 1) // G, (t - 1) % G
                if sl == 0:
                    gt = gpool.tile([128, NCHUNK, G * BLOC], bf16, tag="gt")
                m = mpool.tile([128, NCHUNK * BLOC], f32, tag="m")
                nc.vector._custom_dve(LIF, out=m[:], in0=mem_prev[:], in1=curb[:], s0=BETA)
                nc.scalar.activation(
                    gt[:, :, sl * BLOC:(sl + 1) * BLOC],
                    m[:].rearrange("p (c b) -> p c b", b=BLOC),
                    mybir.ActivationFunctionType.Sign, bias=biasm1[:], scale=1.0,
                )
                mem_prev = m
                if sl == G - 1:
                    pc = psC.tile([128, NO], f32, tag="psc")
                    for c in range(NCHUNK):
                        lhs = gt[:, c, :]
                        nc.tensor.matmul(pc[:], lhs, w2hi_t[:, c, :], start=(c == 0), stop=False)
                        nc.tensor.matmul(pc[:], lhs, w2lo_t[:, c, :], start=False, stop=False)
                    nc.tensor.matmul(pc[:], ones_t[:], b2e_t[:], start=False, stop=True)
                    nc.scalar.activation(
                        cur2buf[:, gi * NO:(gi + 1) * NO], pc[:],
                        mybir.ActivationFunctionType.Copy,
                    )

            # ---------------- rearrange cur2: [sl*32+b, gi*5+o] -> [b, t*5+o]
            cur2r = stage.tile([BLOC, T * NO], f32, tag="cur2r")
            cur2r_v = cur2r[:].rearrange("p (g s o) -> p g s o", s=G, o=NO)
            for sl in range(G):
                nc.sync.dma_start(
                    cur2r_v[:, :, sl, :],
                    cur2buf[sl * BLOC:(sl + 1) * BLOC, :].rearrange("p (g o) -> p g o", o=NO),
                )

            # ---------------- phase D: layer-2 scan
            mem2 = stage.tile([BLOC, T * NO], f32, tag="mem2")
            z32 = stage.tile([BLOC, NO], f32, tag="z32")
            nc.vector.memset(z32[:], 0.0)
            for t in range(T):
                in0 = z32[:] if t == 0 else mem2[:, (t - 1) * NO:t * NO]
                nc.vector._custom_dve(
                    LIF,
                    out=mem2[:, t * NO:(t + 1) * NO],
                    in0=in0,
                    in1=cur2r[:, t * NO:(t + 1) * NO],
                    s0=BETA,
                )
            spk2 = stage.tile([BLOC, T * NO], f32, tag="spk2")
            nc.vector.tensor_scalar(spk2[:], mem2[:], 1.0, None, mybir.AluOpType.is_gt)
            nc.sync.dma_start(mem2_d[:], mem2[:])
            nc.sync.dma_start(spk2_d[:], spk2[:])

    nc.compile()
    _PROGRAMS[sim] = (nc, LIF)
    return _PROGRAMS[sim]


# ---------------------------------------------------------------- host prep
def _prep_inputs(x, W1, b1, W2, b2):
    f32 = np.float32
    x_flat = np.ascontiguousarray(x.reshape(B, -1).astype(f32, copy=False))  # [256, 32000]
    xT = np.zeros((KPAD, B), f32)
    xT[:NIN] = x_flat.T
    xTh = xT.astype(np.float16)
    xTl = (xT - xTh.astype(f32)).astype(np.float16)
    w1T = np.zeros((KPAD, HPAD), f32)
    w1T[:NIN, :NH] = W1.astype(f32, copy=False).T * W1SCALE
    w1Th = w1T.astype(np.float16)
    w1Tl = (w1T - w1Th.astype(f32)).astype(np.float16)
    b1p = np.full(HPAD, -10.0, f32)
    b1p[:NH] = b1
    b1c = np.ascontiguousarray(b1p.reshape(NCHUNK, 128).T)          # [128, 8]
    W2e = np.zeros((HPAD, NO), f32)
    W2e[:NH] = 0.5 * W2.astype(f32, copy=False).T
    w2stack = np.ascontiguousarray(W2e.reshape(NCHUNK, 128, NO).transpose(1, 0, 2))  # [128,8,5]
    w2hi = w2stack.astype(ml_dtypes.bfloat16)
    w2lo = (w2stack - w2hi.astype(f32)).astype(ml_dtypes.bfloat16)
    b2e = (b2.astype(f32) + 0.5 * W2.astype(f32).sum(axis=1)).reshape(1, NO).astype(f32)

    in_maps = []
    for c in range(N_CORES):
        ksl = slice(c * KC, (c + 1) * KC)
        in_maps.append({
            "xth": np.ascontiguousarray(xTh[ksl]).reshape(KTILES, 128, B),
            "xtl": np.ascontiguousarray(xTl[ksl]).reshape(KTILES, 128, B),
            "w1h": np.ascontiguousarray(w1Th[ksl]).reshape(KTILES, 128, HPAD),
            "w1l": np.ascontiguousarray(w1Tl[ksl]).reshape(KTILES, 128, HPAD),
            "b1c": b1c,
            "w2hi": w2hi,
            "w2lo": w2lo,
            "b2e": b2e,
        })
    return in_maps


def _gather(results):
    spk_parts, mem_parts = [], []
    for r in results:
        mem_parts.append(r["mem2rec"].reshape(BLOC, T, NO).transpose(1, 0, 2))
        spk_parts.append(r["spk2rec"].reshape(BLOC, T, NO).transpose(1, 0, 2))
    mem2 = np.concatenate(mem_parts, axis=1).astype(np.float32)  # [200, 256, 5]
    spk2 = np.concatenate(spk_parts, axis=1).astype(np.float32)
    return spk2, mem2


def run_raw(inputs, **kwargs):
    """Build+run; returns BassKernelResults (for profiling from test.py)."""
    from concourse.bass_utils import run_bass_kernel_spmd

    nc, _ = _build_program()
    in_maps = _prep_inputs(**inputs)
    return run_bass_kernel_spmd(nc, in_maps, core_ids=list(range(N_CORES)), **kwargs)


def kernel(x, W1, b1, W2, b2):
    res = run_raw(dict(x=x, W1=W1, b1=b1, W2=W2, b2=b2))
    return _gather(res.results)


if __name__ == "__main__":
    rng = np.random.default_rng(0)
    ins = {
        "x": rng.standard_normal((B, 2, 80, 200)).astype(np.float32),
        "W1": rng.uniform(-1, 1, (NH, NIN)).astype(np.float32) / np.sqrt(NIN),
        "b1": rng.uniform(-1, 1, NH).astype(np.float32) / np.sqrt(NIN),
        "W2": rng.uniform(-1, 1, (NO, NH)).astype(np.float32) / np.sqrt(NH),
        "b2": rng.uniform(-1, 1, NO).astype(np.float32) / np.sqrt(NH),
    }
    spk2, mem2 = kernel(**ins)
    print("shapes:", spk2.shape, mem2.shape, spk2.dtype, mem2.dtype)
    print("spk2 mean:", spk2.mean(), "mem2 std:", mem2.std())



# revision 15
# speedup vs baseline: 1.0237x; 1.0237x over previous
"""Trainium2 Bass kernel for the SNN (LIF) network:

    cur1 = x.reshape(B,-1) @ W1.T + b1          (big fp32 matmul, once)
    200 sequential LIF steps on [B,1000] (layer 1), tiny matmul into 5
    outputs per step (layer 2), second LIF on [B,5].

Distribution over 8 cores (v2, scheduling-optimized):
  Phase A: contraction(K)-sharded exact-fp32 matmul (fp16 hi/lo, 3
           passes), split into two hidden halves; each half's partial
           [256, 512] goes through its own ReduceScatter(add) so the
           collective for half 0 overlaps the matmul of half 1. Each
           core ends with its 32-row batch slice of cur1.
  Phase B: per-core LIF layer-1 scan, hidden on partitions
           ([128, 8 chunks x 32 batch] tiles). One custom DVE
           instruction per step: mem' = beta*mem + cur - (mem > 1).
           Pool engine computes spk = (mem > 1) in bf16 {0,1}.
  Phase C: every 4 steps, PE contracts spk (stationary [128, 4*32])
           against W2 chunks split hi/lo bf16 (exact) into PSUM
           [128(sl,b), 5]; Pool adds b2 into cur2s.
  Phase D: layer-2 LIF steps on [32, 5], interleaved on DVE two groups
           behind phase C. spk2 = (mem2 > 1) on Pool at the end.
"""
import sys

if "/opt/trn_rl_repo" not in sys.path:
    sys.path.insert(0, "/opt/trn_rl_repo")

import numpy as np
import ml_dtypes

# ---------------------------------------------------------------- constants
BETA = 0.95
T = 200
B = 256
NIN = 32000
NH = 1000
NO = 5

N_CORES = 8
KPAD = 32768           # NIN padded to 256*128
KC = KPAD // N_CORES   # 4096 contraction per core
KTILES = KC // 128     # 32
HPAD = 1024            # hidden padded
HHALF = HPAD // 2      # 512 per pipelined half
BLOC = B // N_CORES    # 32 batch rows per core after ReduceScatter
NCHUNK = HPAD // 128   # 8 hidden chunks of 128
G = 4                  # phase-C group size (steps per PE batch)
NGROUP = T // G        # 50
DLAG = 2               # phase-D trails phase-C by this many groups
W1SCALE = 256.0        # W1 pre-scale so the fp16 lo-half stays normal

# ---------------------------------------------------------------- custom op
_LIF_NAME = "LIF_STEP_ANT"


def _register_lif_op():
    from concourse.dve_ops import (
        DveOp, OPS, CUSTOM_DVE_SPECS, _SUB_OPCODE_FOR_NAME, _CUSTOM_DVE_ROW_BASE,
    )
    from concourse.dve_spec import Spec, Src0, Src1, C0, One, lower as dve_lower, _has_src1
    from concourse.dve_uop import DveOpSpec

    for op in OPS:
        if op.name == _LIF_NAME:
            return op
    spec = Spec(
        body=Src0 * C0 + Src1 - (Src0 > One),
        reference=lambda in0, in1, s0: in0 * s0 + in1 - (in0 > 1.0).astype(np.float32),
    )
    if _LIF_NAME not in _SUB_OPCODE_FOR_NAME:
        _SUB_OPCODE_FOR_NAME[_LIF_NAME] = _CUSTOM_DVE_ROW_BASE + len(OPS)
    shas = {}
    for ver in ("v3", "v4"):
        s = DveOpSpec(
            name=_LIF_NAME,
            opcode=_SUB_OPCODE_FOR_NAME[_LIF_NAME],
            uops=dve_lower(spec, ver=ver),
            rd1_en=_has_src1(spec),
        )
        shas[ver] = s.sha(ver)
    op = DveOp(_LIF_NAME, spec, subdim=False, uops_sha=shas)
    OPS.append(op)
    CUSTOM_DVE_SPECS[_LIF_NAME] = op.spec
    return op


# ---------------------------------------------------------------- program
_PROGRAMS = {}  # sim -> (nc, lif_op)


def _build_program(sim=False, dbg=False):
    key = (sim, dbg)
    if key in _PROGRAMS:
        return _PROGRAMS[key]

    import concourse.bass as bass
    import concourse.tile as tile
    from concourse import bacc, mybir
    from concourse.masks import make_identity

    LIF = _register_lif_op()
    f32 = mybir.dt.float32
    bf16 = mybir.dt.bfloat16
    f16 = mybir.dt.float16

    nc = bacc.Bacc("TRN2", target_bir_lowering=False, debug=False,
                   num_devices=1 if sim else N_CORES)

    # inputs (per-core)
    xth_d = nc.dram_tensor("xth", [KTILES, 128, B], f16, kind="ExternalInput").ap()
    xtl_d = nc.dram_tensor("xtl", [KTILES, 128, B], f16, kind="ExternalInput").ap()
    w1h_d = nc.dram_tensor("w1h", [KTILES, 128, HPAD], f16, kind="ExternalInput").ap()
    w1l_d = nc.dram_tensor("w1l", [KTILES, 128, HPAD], f16, kind="ExternalInput").ap()
    b1c_d = nc.dram_tensor("b1c", [128, NCHUNK], f32, kind="ExternalInput").ap()
    w2hi_d = nc.dram_tensor("w2hi", [128, NCHUNK, NO], bf16, kind="ExternalInput").ap()
    w2lo_d = nc.dram_tensor("w2lo", [128, NCHUNK, NO], bf16, kind="ExternalInput").ap()
    b2b_d = nc.dram_tensor("b2b", [128, NO], f32, kind="ExternalInput").ap()
    # outputs (per-core batch slice), free layout = (t, o)
    mem2_d = nc.dram_tensor("mem2rec", [BLOC, T * NO], f32, kind="ExternalOutput").ap()
    spk2_d = nc.dram_tensor("spk2rec", [BLOC, T * NO], f32, kind="ExternalOutput").ap()
    if dbg:
        curdbg_d = nc.dram_tensor("curdbg", [128, NCHUNK * BLOC], f32,
                                  kind="ExternalOutput").ap()
        c2dbg_d = nc.dram_tensor("c2dbg", [BLOC, T * NO], f32,
                                 kind="ExternalOutput").ap()
        gtdbg_d = nc.dram_tensor("gtdbg", [128, NCHUNK * G * BLOC], f32,
                                 kind="ExternalOutput").ap()

    with tile.TileContext(nc) as tc:
        with (
            tc.tile_pool(name="xres", bufs=1) as xres,
            tc.tile_pool(name="win", bufs=4) as wpool,
            tc.tile_pool(name="psA", bufs=2, space="PSUM") as psA,
            tc.tile_pool(name="stage", bufs=1) as stage,
            tc.tile_pool(name="dram", bufs=1, space="DRAM") as dram,
            tc.tile_pool(name="mem", bufs=4) as mpool,
            tc.tile_pool(name="g4", bufs=3) as gpool,
            tc.tile_pool(name="psC", bufs=2, space="PSUM") as psC,
            tc.tile_pool(name="pp", bufs=2) as ppool,
            tc.tile_pool(name="psT", bufs=2, space="PSUM") as psT,
        ):
            # ---------------- stage x resident in SBUF (reused by both halves)
            xall_h = xres.tile([128, KTILES, B], f16, tag="xah")
            xall_l = xres.tile([128, KTILES, B], f16, tag="xal")
            for kt in range(KTILES):
                nc.sync.dma_start(xall_h[:, kt, :], xth_d[kt])
                nc.sync.dma_start(xall_l[:, kt, :], xtl_d[kt])

            # ---------------- phase A (two hidden halves, pipelined with RS)
            partials = []
            rs_outs = []
            for hf in range(2):
                ps = [psA.tile([128, HHALF], f32, tag=f"ps{mb}", name=f"ps{mb}_{hf}")
                      for mb in range(2)]
                hs = slice(hf * HHALF, (hf + 1) * HHALF)
                for kt in range(KTILES):
                    wh_t = wpool.tile([128, HHALF], f16, tag="w1h")
                    nc.sync.dma_start(wh_t[:], w1h_d[kt][:, hs])
                    wl_t = wpool.tile([128, HHALF], f16, tag="w1l")
                    nc.sync.dma_start(wl_t[:], w1l_d[kt][:, hs])
                    last = kt == KTILES - 1
                    for mb in range(2):
                        xh_s = xall_h[:, kt, mb * 128:(mb + 1) * 128]
                        xl_s = xall_l[:, kt, mb * 128:(mb + 1) * 128]
                        out = ps[mb][:]
                        # keep each stationary operand loaded across streams
                        nc.tensor.matmul(out, xh_s, wl_t[:], start=(kt == 0), stop=False)
                        nc.tensor.matmul(out, xh_s, wh_t[:], start=False, stop=False)
                        nc.tensor.matmul(out, xl_s, wh_t[:], start=False, stop=last)
                partial = dram.tile([B, HHALF], f32, tag=f"partial{hf}")
                for mb in range(2):
                    cs = stage.tile([128, HHALF], f32, tag=f"curp{mb}{hf}")
                    nc.scalar.activation(cs[:], ps[mb][:],
                                         mybir.ActivationFunctionType.Copy,
                                         scale=1.0 / W1SCALE)
                    nc.sync.dma_start(partial[mb * 128:(mb + 1) * 128, :], cs[:])
                rs_out = dram.tile([BLOC, HHALF], f32, tag=f"rs{hf}")
                if sim:
                    nc.sync.dma_start(rs_out[:], partial[0:BLOC, :])
                else:
                    nc.gpsimd.collective_compute(
                        "ReduceScatter",
                        mybir.AluOpType.add,
                        replica_groups=[list(range(N_CORES))],
                        ins=[partial.opt()],
                        outs=[rs_out.opt()],
                    )
                partials.append(partial)
                rs_outs.append(rs_out)

            # ---------------- transpose to scan layout + fold b1
            # curb[p, c*32 + b] = cur1[b, c*128 + p] + b1[c*128 + p]
            ident = stage.tile([BLOC, BLOC], f32, tag="ident")
            make_identity(nc, ident[:])
            b1t = stage.tile([128, NCHUNK], f32, tag="b1t")
            nc.sync.dma_start(b1t[:], b1c_d[:])
            rsb = [stage.tile([BLOC, HHALF], f32, tag=f"rsb{hf}", name=f"rsb{hf}")
                   for hf in range(2)]
            for hf in range(2):
                nc.sync.dma_start(rsb[hf][:], rs_outs[hf][:])
            curb = stage.tile([128, NCHUNK * BLOC], f32, tag="curb")
            for c in range(NCHUNK):
                hf, ci = divmod(c, NCHUNK // 2)
                pt = psT.tile([128, BLOC], f32, tag="pst")
                nc.tensor.transpose(pt[:], rsb[hf][:, ci * 128:(ci + 1) * 128], ident[:])
                nc.scalar.activation(
                    curb[:, c * BLOC:(c + 1) * BLOC], pt[:],
                    mybir.ActivationFunctionType.Identity,
                    bias=b1t[:, c:c + 1], scale=1.0,
                )

            if dbg:
                nc.sync.dma_start(curdbg_d[:], curb[:])

            # ---------------- scan constants
            w2hi_t = stage.tile([128, NCHUNK, NO], bf16, tag="w2hi")
            nc.sync.dma_start(w2hi_t[:], w2hi_d[:])
            w2lo_t = stage.tile([128, NCHUNK, NO], bf16, tag="w2lo")
            nc.sync.dma_start(w2lo_t[:], w2lo_d[:])
            b2b_t = stage.tile([128, NO], f32, tag="b2b")
            nc.sync.dma_start(b2b_t[:], b2b_d[:])
            biasm1 = stage.tile([128, 1], f32, tag="bm1")
            nc.vector.memset(biasm1[:], -1.0)
            zeros_t = stage.tile([128, NCHUNK * BLOC], f32, tag="zeros")
            nc.vector.memset(zeros_t[:], 0.0)
            z32 = stage.tile([BLOC, NO], f32, tag="z32")
            nc.vector.memset(z32[:], 0.0)
            c2r = stage.tile([BLOC, T * NO], f32, tag="c2r")
            mem2r = stage.tile([BLOC, T * NO], f32, tag="mem2r")
            spk2r = stage.tile([BLOC, T * NO], f32, tag="spk2r")

            def d_step(dt):
                """Layer-2 LIF step dt (0-based) on DVE, [32, 5]."""
                in0 = z32[:] if dt == 0 else mem2r[:, (dt - 1) * NO:dt * NO]
                nc.vector._custom_dve(
                    LIF,
                    out=mem2r[:, dt * NO:(dt + 1) * NO],
                    in0=in0,
                    in1=c2r[:, dt * NO:(dt + 1) * NO],
                    s0=BETA,
                )

            # ---------------- phase B/C/D: fused scan
            mem_prev = zeros_t
            gt = None
            for t in range(1, T + 1):
                gi, sl = (t - 1) // G, (t - 1) % G
                if sl == 0:
                    gt = gpool.tile([128, NCHUNK, G * BLOC], bf16, tag="gt")
                m = mpool.tile([128, NCHUNK * BLOC], f32, tag="m")
                nc.vector._custom_dve(LIF, out=m[:], in0=mem_prev[:], in1=curb[:], s0=BETA)
                # g = sign(mem - 1) in {-1,+1} bf16 on ACT; spk=(1+g)/2 folded
                # into the 0.5-scaled W2 and b2eff on the host.
                nc.scalar.activation(
                    gt[:, :, sl * BLOC:(sl + 1) * BLOC],
                    m[:].rearrange("p (c b) -> p c b", b=BLOC),
                    mybir.ActivationFunctionType.Sign, bias=biasm1[:], scale=1.0,
                )
                mem_prev = m
                if sl == G - 1:
                    pc = psC.tile([128, NO], f32, tag="psc")
                    for c in range(NCHUNK):
                        lhs = gt[:, c, :]
                        nc.tensor.matmul(pc[:], lhs, w2hi_t[:, c, :], start=(c == 0), stop=False)
                        nc.tensor.matmul(pc[:], lhs, w2lo_t[:, c, :], start=False,
                                         stop=(c == NCHUNK - 1))
                    # GpSimd can't read PSUM: ACT copies out, GpSimd adds b2eff
                    pcs = ppool.tile([128, NO], f32, tag="pcs")
                    nc.scalar.activation(pcs[:], pc[:],
                                         mybir.ActivationFunctionType.Copy)
                    pcb = ppool.tile([128, NO], f32, tag="pcb")
                    nc.gpsimd.tensor_tensor(
                        pcb[:], pcs[:], b2b_t[:], mybir.AluOpType.add,
                    )
                    # custom-DVE in1 can't take a partition offset: DMA each
                    # sl-row block down to partition base 0 in (t, o) layout
                    for s2 in range(G):
                        dt2 = gi * G + s2
                        nc.sync.dma_start(
                            c2r[:, dt2 * NO:(dt2 + 1) * NO],
                            pcb[s2 * BLOC:(s2 + 1) * BLOC, :],
                        )
                    if dbg and gi == 0:
                        gtf = stage.tile([128, NCHUNK * G * BLOC], f32, tag="gtf")
                        nc.vector.tensor_copy(
                            gtf[:], gt[:].rearrange("p c s -> p (c s)"))
                        nc.sync.dma_start(gtdbg_d[:], gtf[:])
                    if gi >= DLAG:
                        for dt in range((gi - DLAG) * G, (gi - DLAG + 1) * G):
                            d_step(dt)
            for dt in range((NGROUP - DLAG) * G, T):
                d_step(dt)

            # ---------------- spk2 + outputs
            if dbg:
                nc.sync.dma_start(c2dbg_d[:], c2r[:])
            nc.vector.tensor_scalar(spk2r[:], mem2r[:], 1.0, None, mybir.AluOpType.is_gt)
            nc.sync.dma_start(mem2_d[:], mem2r[:])
            nc.sync.dma_start(spk2_d[:], spk2r[:])

    nc.compile()
    _PROGRAMS[key] = (nc, LIF)
    return _PROGRAMS[key]


# ---------------------------------------------------------------- host prep
def _prep_inputs(x, W1, b1, W2, b2):
    f32 = np.float32
    x_flat = np.ascontiguousarray(x.reshape(B, -1).astype(f32, copy=False))  # [256, 32000]
    xT = np.zeros((KPAD, B), f32)
    xT[:NIN] = x_flat.T
    xTh = xT.astype(np.float16)
    xTl = (xT - xTh.astype(f32)).astype(np.float16)
    w1T = np.zeros((KPAD, HPAD), f32)
    w1T[:NIN, :NH] = W1.astype(f32, copy=False).T * W1SCALE
    w1Th = w1T.astype(np.float16)
    w1Tl = (w1T - w1Th.astype(f32)).astype(np.float16)
    b1p = np.full(HPAD, -10.0, f32)
    b1p[:NH] = b1
    b1c = np.ascontiguousarray(b1p.reshape(NCHUNK, 128).T)          # [128, 8]
    W2e = np.zeros((HPAD, NO), f32)
    W2e[:NH] = 0.5 * W2.astype(f32, copy=False).T
    w2stack = np.ascontiguousarray(W2e.reshape(NCHUNK, 128, NO).transpose(1, 0, 2))  # [128,8,5]
    w2hi = w2stack.astype(ml_dtypes.bfloat16)
    w2lo = (w2stack - w2hi.astype(f32)).astype(ml_dtypes.bfloat16)
    b2eff = (b2.astype(f32) + 0.5 * W2.astype(f32).sum(axis=1)).reshape(1, NO)
    b2b = np.ascontiguousarray(np.tile(b2eff, (128, 1)).astype(f32))

    in_maps = []
    for c in range(N_CORES):
        ksl = slice(c * KC, (c + 1) * KC)
        in_maps.append({
            "xth": np.ascontiguousarray(xTh[ksl]).reshape(KTILES, 128, B),
            "xtl": np.ascontiguousarray(xTl[ksl]).reshape(KTILES, 128, B),
            "w1h": np.ascontiguousarray(w1Th[ksl]).reshape(KTILES, 128, HPAD),
            "w1l": np.ascontiguousarray(w1Tl[ksl]).reshape(KTILES, 128, HPAD),
            "b1c": b1c,
            "w2hi": w2hi,
            "w2lo": w2lo,
            "b2b": b2b,
        })
    return in_maps


def _gather(results):
    spk_parts, mem_parts = [], []
    for r in results:
        mem_parts.append(r["mem2rec"].reshape(BLOC, T, NO).transpose(1, 0, 2))
        spk_parts.append(r["spk2rec"].reshape(BLOC, T, NO).transpose(1, 0, 2))
    mem2 = np.concatenate(mem_parts, axis=1).astype(np.float32)  # [200, 256, 5]
    spk2 = np.concatenate(spk_parts, axis=1).astype(np.float32)
    return spk2, mem2


def run_raw(inputs, dbg=False, **kwargs):
    """Build+run; returns BassKernelResults (for profiling from test.py)."""
    from concourse.bass_utils import run_bass_kernel_spmd

    nc, _ = _build_program(dbg=dbg)
    in_maps = _prep_inputs(**inputs)
    return run_bass_kernel_spmd(nc, in_maps, core_ids=list(range(N_CORES)), **kwargs)


def kernel(x, W1, b1, W2, b2):
    res = run_raw(dict(x=x, W1=W1, b1=b1, W2=W2, b2=b2))
    return _gather(res.results)


if __name__ == "__main__":
    rng = np.random.default_rng(0)
    ins = {
        "x": rng.standard_normal((B, 2, 80, 200)).astype(np.float32),
        "W1": rng.uniform(-1, 1, (NH, NIN)).astype(np.float32) / np.sqrt(NIN),
        "b1": rng.uniform(-1, 1, NH).astype(np.float32) / np.sqrt(NIN),
        "W2": rng.uniform(-1, 1, (NO, NH)).astype(np.float32) / np.sqrt(NH),
        "b2": rng.uniform(-1, 1, NO).astype(np.float32) / np.sqrt(NH),
    }
    spk2, mem2 = kernel(**ins)
    print("shapes:", spk2.shape, mem2.shape, spk2.dtype, mem2.dtype)
    print("spk2 mean:", spk2.mean(), "mem2 std:", mem2.std())
